# revision 1
# baseline (speedup 1.0000x reference)
"""Trainium2 Bass kernel for nn_ContrastiveLoss (SimCLR-style NT-Xent loss).

Reference computation:
    f = normalize(concat([z1, z2]))            # [2B, D] unit rows
    S = f @ f.T / T                            # [8192, 8192]
    loss = mean_i( logsumexp_j(S[i, :]) - S[i, pos_i] )

Sharding: each of the 8 cores owns a 1024-row block of S and computes it
against all 8192 columns (all-gathered features), then does a local
sum-of-exp per row plus the positive-pair dot for its rows. The final
(tiny) reduction over rows/cores happens on the host in float64.

Device-side plan per core:
  1. Cast-load z as [128, 4, 512] bf16 super-tiles (gpsimd DMA cast, four
     row-tiles per DMA), fused square+row-sum per row tile
     (DVE affine_mul_reduce), inverse norms via Quake-style Newton rsqrt
     entirely on DVE (no ACT table switches), unit-scale rows
     (DVE tensor_scalar).
  2. Stage normalized bf16 rows to DRAM, transpose via DMA-xbar into
     FT[d, j] layout (contraction dim on partitions).
  3. 512 bf16 matmuls [128k x 128m] @ [128k x 512n] accumulate row-blocks
     of cos-sim into PSUM; ACT computes exp(x/T) in place on PSUM with a
     fused row-sum (accum_out). Column-block-outer loop order so the PE
     only needs the first transposed group to start.
  4. Positive term: fused dots of the core's own rows with their pair
     rows (separately sliced per-core inputs zme/zpos).

Outputs per core: "sums" [128, 32] (partial exp-sums per row, 2048-col
chunks) and "pos" [128, 8] (cosine of positive pairs). Host: loss =
mean(log(sum(sums)) - pos/T).

The max-subtraction in the reference logsumexp is skipped on device:
|cos|/T <= ~14.7, so sum_j exp() <= ~8192*e^14.7 ~ 2e10, well within
fp32 range.
"""

import os
import sys

try:
    import concourse.bass  # noqa: F401
except ImportError:
    for _p in ("/root/.axon_site/_ro/trn_rl_repo", "/opt/trn_rl_repo"):
        if _p not in sys.path and os.path.isdir(_p):
            sys.path.insert(0, _p)

import numpy as np

B = 4096
D = 512
T = 0.07
P = 128
NCORES = 8
R = (2 * B) // NCORES  # 1024 rows per core / per group
RT = R // P            # 8 row tiles per core
G = (2 * B) // R       # 8 groups of columns
GT = R // P            # 8 row tiles per group
ST = 4                 # row tiles per cast-load super-tile
NSUP = GT // ST        # super-tiles per group
DC = D // P            # 4 contraction chunks of 128
PSW = 1024             # psum tile width (2 banks); one ft group per C block
NB = (2 * B) // PSW    # 4 psum tiles per row tile
NS = PSW // 512        # 4 matmul n-slices per psum tile

_NC = None


def _build():
    from contextlib import ExitStack

    import concourse.bacc as bacc
    import concourse.tile as tile
    from concourse import mybir

    f32 = mybir.dt.float32
    bf16 = mybir.dt.bfloat16
    i32 = mybir.dt.int32
    AFT = mybir.ActivationFunctionType
    EXPF = AFT.Exp
    MUL = mybir.AluOpType.mult
    ADD = mybir.AluOpType.add
    SUB = mybir.AluOpType.subtract
    SHR = mybir.AluOpType.logical_shift_right

    nc = bacc.Bacc(
        "TRN2", target_bir_lowering=False, debug=False, num_devices=NCORES
    )
    z1 = nc.dram_tensor("z1", [B, D], f32, kind="ExternalInput")
    z2 = nc.dram_tensor("z2", [B, D], f32, kind="ExternalInput")
    zme = nc.dram_tensor("zme", [R, D], f32, kind="ExternalInput")
    zpos = nc.dram_tensor("zpos", [R, D], f32, kind="ExternalInput")
    sums_out = nc.dram_tensor("sums", [P, RT * NB], f32, kind="ExternalOutput")
    pos_out = nc.dram_tensor("pos", [P, RT], f32, kind="ExternalOutput")

    with ExitStack() as ctx:
        tc = ctx.enter_context(tile.TileContext(nc))
        smalls = ctx.enter_context(tc.tile_pool(name="smalls", bufs=1))
        dumps = ctx.enter_context(tc.tile_pool(name="dumps", bufs=4))
        stats = ctx.enter_context(tc.tile_pool(name="stats", bufs=3))
        apool = ctx.enter_context(tc.tile_pool(name="apool", bufs=NSUP))
        zbpool = ctx.enter_context(tc.tile_pool(name="zbpool", bufs=NSUP + 2))
        fnpool = ctx.enter_context(tc.tile_pool(name="fnpool", bufs=3))
        ftpool = ctx.enter_context(tc.tile_pool(name="ftpool", bufs=1))
        dram = ctx.enter_context(tc.tile_pool(name="stage", bufs=1, space="DRAM"))
        psum = ctx.enter_context(tc.tile_pool(name="psum", bufs=4, space="PSUM"))

        sums_sb = smalls.tile([P, RT * NB], f32, tag="sums_sb")
        pos_sb = smalls.tile([P, RT], f32, tag="pos_sb")
        posraw = smalls.tile([P, RT], f32, tag="posraw")
        invn_me = smalls.tile([P, GT], f32, tag="invn_me")
        invn_pos = smalls.tile([P, GT], f32, tag="invn_pos")
        magic = smalls.tile([P, GT], i32, tag="magic")
        nc.vector.memset(magic[:], 0x5F3759DF)

        def mulsum(in0, in1, accum_col):
            # accum_col[p] = sum_x in0[p,x]*in1[p,x] in one DVE op; the
            # mandatory main output goes to a throwaway broadcast AP.
            # (The ISA TENSOR_TENSOR_REDUCE op hard-crashes this runtime;
            # the custom-DVE affine_mul_reduce is the working equivalent.)
            dummy = dumps.tile([P, 1], f32, tag="dummy")
            nc.vector.affine_mul_reduce(
                out=dummy.broadcast_to(in0.shape),
                accum_out=accum_col,
                in0=in0,
                in1=in1,
                scale=1.0,
                bias=0.0,
            )

        def load_group(src, pool, tag, ssq):
            """Cast-load [R, D] fp32 rows as NSUP bf16 super-tiles
            [P, ST, D] (row tile t = [:, t%ST, :] of super t//ST) and
            accumulate per-row-tile sum-of-squares into ssq [P, GT]."""
            sup = []
            for s in range(NSUP):
                zb = pool.tile([P, ST, D], bf16, tag=tag)
                nc.gpsimd.dma_start(
                    out=zb[:],
                    in_=src[s * ST * P : (s + 1) * ST * P, :].rearrange(
                        "(a p) d -> p a d", p=P
                    ),
                )
                for a in range(ST):
                    t = s * ST + a
                    mulsum(zb[:, a, :], zb[:, a, :], ssq[:, t : t + 1])
                sup.append(zb)
            return sup

        def rsqrt(invn_dst, ssq):
            # 1/max(sqrt(s), eps) == min(rsqrt(s), 1e12); rsqrt via the
            # Quake bit-trick + 2 Newton iterations, all on DVE — keeps
            # ACT's function-table set pinned to Exp for the whole kernel.
            h = stats.tile([P, GT], i32, tag="h")
            nc.vector.tensor_scalar(h[:], ssq[:].bitcast(i32), 1, None, op0=SHR)
            y = stats.tile([P, GT], f32, tag="y")
            nc.vector.tensor_tensor(y[:].bitcast(i32), magic[:], h[:], op=SUB)
            a = stats.tile([P, GT], f32, tag="a")
            for _ in range(2):
                nc.vector.tensor_mul(a[:], y[:], y[:])
                nc.vector.tensor_mul(a[:], a[:], ssq[:])
                nc.vector.tensor_scalar(a[:], a[:], -0.5, 1.5, op0=MUL, op1=ADD)
                nc.vector.tensor_mul(y[:], y[:], a[:])
            nc.vector.tensor_scalar_min(invn_dst, y[:], 1.0e12)

        def norm_group(src, pool, tag, invn_dst):
            ssq = stats.tile([P, GT], f32, tag="ssq")
            sup = load_group(src, pool, tag, ssq)
            rsqrt(invn_dst, ssq)
            return sup

        def scale_stage_transpose(sup, invn, stage_tag, ft_tags):
            # Staging writes go through the ACT HWDGE (nc.scalar) so the SP
            # HWDGE FIFO carries only the xbar transposes — otherwise a
            # group's transposes queue behind later groups' stage writes.
            stg = dram.tile([R, D], bf16, tag=stage_tag)
            for s in range(NSUP):
                fn = fnpool.tile([P, ST, D], bf16, tag="fn")
                for a in range(ST):
                    t = s * ST + a
                    nc.vector.tensor_scalar_mul(
                        fn[:, a, :], sup[s][:, a, :], invn[:, t : t + 1]
                    )
                rows = slice(s * ST * P, (s + 1) * ST * P)
                nc.scalar.dma_start(
                    out=stg[rows, :].rearrange("(a p) d -> p a d", p=P),
                    in_=fn[:],
                )
            fts = []
            for dc in range(DC):
                fth = ftpool.tile([P, R], bf16, tag=ft_tags(dc), name=ft_tags(dc))
                nc.sync.dma_start(
                    out=fth[:], in_=stg[:, dc * P : (dc + 1) * P], transpose=True
                )
                fts.append(fth)
            return fts

        def build_group(g):
            src = z1 if g < G // 2 else z2
            g0 = (g % (G // 2)) * R
            invn_g = stats.tile([P, GT], f32, tag="invn")
            sup = norm_group(src[g0 : g0 + R, :], zbpool, "zb", invn_g[:])
            return scale_stage_transpose(
                sup, invn_g[:], f"stg{g}", lambda dc: f"ft{g}_{dc}"
            )

        def sim_block(nb, ftm, ft):
            last_exp = None
            for r in range(RT):
                ps = psum.tile([P, PSW], f32, tag="ps")
                for dc in range(DC):
                    for ns in range(NS):
                        j0 = nb * PSW + ns * 512
                        gj, cj = divmod(j0, R)
                        nc.tensor.matmul(
                            ps[:, ns * 512 : (ns + 1) * 512],
                            ftm[dc][:, r * P : (r + 1) * P],
                            ft[gj][dc][:, cj : cj + 512],
                            start=(dc == 0),
                            stop=(dc == DC - 1),
                        )
                last_exp = nc.scalar.activation(
                    ps[:],
                    ps[:],
                    EXPF,
                    scale=1.0 / T,
                    accum_out=sums_sb[:, r * NB + nb : r * NB + nb + 1],
                )
            return last_exp

        # Program order doubles as scheduler priority: own rows (lhsT) first,
        # then each column super-block's two ft groups right before the
        # matmuls that consume them; remaining groups' loads gap-fill under
        # PE work. The zpos/pos-term work is independent of the matmul
        # pipeline and goes last.
        zme_t = norm_group(zme, apool, "zme", invn_me[:])
        ftm = scale_stage_transpose(zme_t, invn_me[:], "stgme", lambda dc: f"ftm{dc}")
        ft = [None] * G
        anchors = []
        for nb in range(NB):
            ft[nb] = build_group(nb)
            anchors.append(sim_block(nb, ftm, ft))

        # The zpos/pos-term chain is off the matmul critical path; pin its
        # loads behind block 1 so they can't gap-fill into the prologue and
        # steal DMA/DVE bandwidth from the ftm/ft[0] chain.
        from concourse.tile import add_dep_helper

        _zpos_deps = [anchors[1].ins]
        _old_gp_dma = nc.gpsimd.dma_start

        def _dep_dma(*a, **k):
            inst = _old_gp_dma(*a, **k)
            for d in _zpos_deps:
                add_dep_helper(inst.ins, d, reason="delay zpos past prologue")
            return inst

        nc.gpsimd.dma_start = _dep_dma
        zpos_t = norm_group(zpos, apool, "zpos", invn_pos[:])
        nc.gpsimd.dma_start = _old_gp_dma
        for s in range(NSUP):
            for a in range(ST):
                t = s * ST + a
                mulsum(zme_t[s][:, a, :], zpos_t[s][:, a, :], posraw[:, t : t + 1])
        nc.vector.tensor_mul(pos_sb[:], posraw[:], invn_me[:])
        nc.vector.tensor_mul(pos_sb[:], pos_sb[:], invn_pos[:])

        nc.sync.dma_start(out=sums_out[:], in_=sums_sb[:])
        nc.sync.dma_start(out=pos_out[:], in_=pos_sb[:])

    nc.compile()
    return nc


def _get_nc():
    global _NC
    if _NC is None:
        _NC = _build()
    return _NC


def run(z1, z2, trace=False):
    """Run the SPMD kernel; returns (loss, BassKernelResults)."""
    from concourse.bass_utils import run_bass_kernel_spmd

    z1 = np.ascontiguousarray(z1, dtype=np.float32)
    z2 = np.ascontiguousarray(z2, dtype=np.float32)
    in_maps = []
    for c in range(NCORES):
        if c < NCORES // 2:
            zme_c, zpos_c = z1[c * R : (c + 1) * R], z2[c * R : (c + 1) * R]
        else:
            c2 = c - NCORES // 2
            zme_c, zpos_c = z2[c2 * R : (c2 + 1) * R], z1[c2 * R : (c2 + 1) * R]
        in_maps.append(
            {
                "z1": z1,
                "z2": z2,
                "zme": np.ascontiguousarray(zme_c),
                "zpos": np.ascontiguousarray(zpos_c),
            }
        )
    res = run_bass_kernel_spmd(
        _get_nc(), in_maps, core_ids=list(range(NCORES)), trace=trace
    )
    total = 0.0
    for r in res.results:
        sums = r["sums"].astype(np.float64)  # [P, RT*NB] partial exp-sums
        pos = r["pos"].astype(np.float64)    # [P, RT] positive-pair cosines
        sumexp = sums.reshape(P, RT, NB).sum(axis=2)
        total += (np.log(sumexp) - pos / T).sum()
    loss = total / (2.0 * B)
    return np.float32(loss), res


def kernel(z1, z2, labels=None, **_ignored):
    loss, _ = run(z1, z2, trace=False)
    return np.asarray(loss, dtype=np.float32)


if __name__ == "__main__":
    rng = np.random.default_rng(0)
    a = rng.standard_normal((B, D)).astype(np.float32)
    b = rng.standard_normal((B, D)).astype(np.float32)
    print(kernel(a, b, None))



# revision 8
# speedup vs baseline: 1.6188x; 1.6188x over previous
"""Trainium2 Bass kernel for nn_ContrastiveLoss (SimCLR-style NT-Xent loss).

Reference computation:
    f = normalize(concat([z1, z2]))            # [2B, D] unit rows
    S = f @ f.T / T                            # [8192, 8192]
    loss = mean_i( logsumexp_j(S[i, :]) - S[i, pos_i] )

Sharding exploits S's symmetry: the 8x8 grid of 1024x1024 blocks has 36
distinct blocks (8 diagonal + 28 unordered off-diagonal pairs).  Core c
computes its diagonal block, the full blocks (c, c+1..c+3 mod 8), and two
512x512 quadrants of the distance-4 pair {c, c+4} -- 4.5 block-equivalents
(4608 columns) instead of 8, a 1.78x cut in matmul/exp work.  Row-sums of
exp come from ACT's fused accumulator; the transposed contributions (the
blocks this core does NOT compute) are recovered as COLUMN-sums of the
computed exp tiles via ones-vector matmuls accumulated in PSUM.  The host
combines row-sums, col-sums and positive-pair cosines in float64.

The program is identical on all cores (SPMD); the block assignment is
data-driven: core c's "zc" input holds bands (c..c+4 mod 8) of raw rows,
and for c>=4 the distance-4 band has its 512-row halves swapped so the
quadrant split covers each {c, c+4} entry exactly once:
  core c   computes  own[0:512] x b4[0:512]  and  own[512:] x b4[512:]
  core c+4 sees band c half-swapped, so the same program yields the two
  complementary quadrants.

Device-side plan per core:
  1. Cast-load each band as [128, 4, 512] bf16 super-tiles, fused
     square+row-sum (DVE affine_mul_reduce), Newton rsqrt on DVE (keeps
     ACT's table pinned to Exp), unit-scale rows, stage bf16 rows to
     DRAM, DMA-xbar transpose into FT[d, j] chunks.
  2. Per column band k (own rows stationary, weight-reuse order):
     8 row tiles x 4 dc x (2|1) n-slices of bf16 matmuls accumulate
     S-block rows into PSUM; ACT computes exp(x/T) with fused row-sum
     (accum_out), writing bf16 exp tiles to SBUF for k>=1.
  3. Col-sums: ones[128,1] matmuls over the exp tiles accumulate
     sum_i exp(S[i, j]) per column into [1, 512] PSUM groups across row
     tiles; copied out to an SBUF strip.
  4. Positive term: fused dots of normalized own rows with normalized
     zpos rows (cosines directly).

Outputs per core: "sums" [128, 40] (row exp-sums per (row tile, k)),
"csums" [1, 4096] (col-sum strips for k=1..4), "pos" [128, 8].

The max-subtraction in the reference logsumexp is skipped on device:
|cos|/T <= ~14.7 so all partial sums stay well within fp32 range.
"""

import os
import sys

try:
    import concourse.bass  # noqa: F401
except ImportError:
    for _p in ("/root/.axon_site/_ro/trn_rl_repo", "/opt/trn_rl_repo"):
        if _p not in sys.path and os.path.isdir(_p):
            sys.path.insert(0, _p)

import numpy as np

B = 4096
D = 512
T = 0.07
P = 128
NCORES = 8
R = (2 * B) // NCORES  # 1024 rows per band
RT = R // P            # 8 row tiles per band
NBANDS = 5             # column bands per core: own + next 3 full + dist-4 half
ST = 4                 # row tiles per cast-load super-tile
NSUP = RT // ST        # super-tiles per band
DC = D // P            # 4 contraction chunks of 128
NK = 5                 # k = 0 (diag) .. 4 (quadrants)

_NC = None


def _build():
    from contextlib import ExitStack

    import concourse.bacc as bacc
    import concourse.tile as tile
    from concourse import mybir

    f32 = mybir.dt.float32
    bf16 = mybir.dt.bfloat16
    i32 = mybir.dt.int32
    AFT = mybir.ActivationFunctionType
    EXPF = AFT.Exp
    MUL = mybir.AluOpType.mult
    ADD = mybir.AluOpType.add
    SUB = mybir.AluOpType.subtract
    SHR = mybir.AluOpType.logical_shift_right

    nc = bacc.Bacc(
        "TRN2", target_bir_lowering=False, debug=False, num_devices=NCORES
    )
    zc = nc.dram_tensor("zc", [NBANDS * R, D], f32, kind="ExternalInput")
    zpos = nc.dram_tensor("zpos", [R, D], f32, kind="ExternalInput")
    sums_out = nc.dram_tensor("sums", [P, RT * NK], f32, kind="ExternalOutput")
    csums_out = nc.dram_tensor("csums", [1, 4 * R], f32, kind="ExternalOutput")
    pos_out = nc.dram_tensor("pos", [P, RT], f32, kind="ExternalOutput")

    with ExitStack() as ctx:
        tc = ctx.enter_context(tile.TileContext(nc))
        smalls = ctx.enter_context(tc.tile_pool(name="smalls", bufs=1))
        dumps = ctx.enter_context(tc.tile_pool(name="dumps", bufs=4))
        stats = ctx.enter_context(tc.tile_pool(name="stats", bufs=3))
        zbpool = ctx.enter_context(tc.tile_pool(name="zbpool", bufs=4))
        fnpool = ctx.enter_context(tc.tile_pool(name="fnpool", bufs=3))
        keepfn = ctx.enter_context(tc.tile_pool(name="keepfn", bufs=2))
        ftpool = ctx.enter_context(tc.tile_pool(name="ftpool", bufs=1))
        exppool = ctx.enter_context(tc.tile_pool(name="exppool", bufs=2))
        dram = ctx.enter_context(tc.tile_pool(name="stage", bufs=1, space="DRAM"))
        psum = ctx.enter_context(tc.tile_pool(name="psum", bufs=3, space="PSUM"))
        cspsum = ctx.enter_context(tc.tile_pool(name="cspsum", bufs=1, space="PSUM"))

        sums_sb = smalls.tile([P, RT * NK], f32, tag="sums_sb")
        csums_sb = smalls.tile([1, 4 * R], f32, tag="csums_sb")
        pos_sb = smalls.tile([P, RT], f32, tag="pos_sb")
        magic = smalls.tile([P, RT], i32, tag="magic")
        ones = smalls.tile([P, 1], bf16, tag="ones")
        nc.vector.memset(magic[:], 0x5F3759DF)
        nc.vector.memset(ones[:], 1.0)

        def mulsum(in0, in1, accum_col):
            # accum_col[p] = sum_x in0[p,x]*in1[p,x] in one DVE op; the
            # mandatory main output goes to a throwaway broadcast AP.
            # (The ISA TENSOR_TENSOR_REDUCE op hard-crashes this runtime;
            # the custom-DVE affine_mul_reduce is the working equivalent.)
            dummy = dumps.tile([P, 1], f32, tag="dummy")
            nc.vector.affine_mul_reduce(
                out=dummy.broadcast_to(in0.shape),
                accum_out=accum_col,
                in0=in0,
                in1=in1,
                scale=1.0,
                bias=0.0,
            )

        def rsqrt(invn_dst, ssq):
            # 1/max(sqrt(s), eps) == min(rsqrt(s), 1e12); rsqrt via the
            # Quake bit-trick + 2 Newton iterations, all on DVE — keeps
            # ACT's function-table set pinned to Exp for the whole kernel.
            h = stats.tile([P, RT], i32, tag="h")
            nc.vector.tensor_scalar(h[:], ssq[:].bitcast(i32), 1, None, op0=SHR)
            y = stats.tile([P, RT], f32, tag="y")
            nc.vector.tensor_tensor(y[:].bitcast(i32), magic[:], h[:], op=SUB)
            a = stats.tile([P, RT], f32, tag="a")
            for _ in range(2):
                nc.vector.tensor_mul(a[:], y[:], y[:])
                nc.vector.tensor_mul(a[:], a[:], ssq[:])
                nc.vector.tensor_scalar(a[:], a[:], -0.5, 1.5, op0=MUL, op1=ADD)
                nc.vector.tensor_mul(y[:], y[:], a[:])
            nc.vector.tensor_scalar_min(invn_dst, y[:], 1.0e12)

        def load_norm(src, tag, fpool, ftag):
            """Cast-load [R, D] fp32 rows as NSUP bf16 super-tiles
            [P, ST, D], unit-scale rows; returns list of scaled
            super-tiles (bf16) allocated from fpool/ftag."""
            ssq = stats.tile([P, RT], f32, tag=f"ssq_{tag}")
            sup = []
            for s in range(NSUP):
                zb = zbpool.tile([P, ST, D], bf16, tag="zb")
                nc.gpsimd.dma_start(
                    out=zb[:],
                    in_=src[s * ST * P : (s + 1) * ST * P, :].rearrange(
                        "(a p) d -> p a d", p=P
                    ),
                )
                for a in range(ST):
                    t = s * ST + a
                    mulsum(zb[:, a, :], zb[:, a, :], ssq[:, t : t + 1])
                sup.append(zb)
            invn = stats.tile([P, RT], f32, tag=f"invn_{tag}")
            rsqrt(invn[:], ssq)
            fns = []
            for s in range(NSUP):
                fn = fpool.tile([P, ST, D], bf16, tag=ftag)
                for a in range(ST):
                    t = s * ST + a
                    nc.vector.tensor_scalar_mul(
                        fn[:, a, :], sup[s][:, a, :], invn[:, t : t + 1]
                    )
                fns.append(fn)
            return fns

        def stage_transpose(fns, g):
            # Staging writes go through the ACT HWDGE (nc.scalar) so the SP
            # HWDGE FIFO carries only the xbar transposes.
            stg = dram.tile([R, D], bf16, tag=f"stg{g}")
            for s in range(NSUP):
                rows = slice(s * ST * P, (s + 1) * ST * P)
                nc.scalar.dma_start(
                    out=stg[rows, :].rearrange("(a p) d -> p a d", p=P),
                    in_=fns[s][:],
                )
            fts = []
            for dc in range(DC):
                fth = ftpool.tile(
                    [P, R], bf16, tag=f"ft{g}_{dc}", name=f"ft{g}_{dc}"
                )
                # Two half transposes so the first can start as soon as the
                # first super-tile is staged (shorter critical path).
                for h in range(2):
                    nc.sync.dma_start(
                        out=fth[:, h * 512 : (h + 1) * 512],
                        in_=stg[h * 512 : (h + 1) * 512, dc * P : (dc + 1) * P],
                        transpose=True,
                    )
                fts.append(fth)
            return fts

        def build_band(g, fpool=fnpool, ftag="fn"):
            fns = load_norm(zc[g * R : (g + 1) * R, :], f"b{g}", fpool, ftag)
            return fns, stage_transpose(fns, g)

        # ---- own band first: its FT chunks are the stationary operands ----
        # Own fns persist (tag "fno") for the positive-pair dots at the end.
        own_fns, own_ft = build_band(0, keepfn, "fno")
        ft = [None] * NBANDS
        ft[0] = own_ft

        anchors = {}

        def emit_k(k, ftk):
            anchor = None
            if k >= 1:
                cs_a = cspsum.tile([1, 512], f32, tag="cs_a")
                cs_b = cspsum.tile([1, 512], f32, tag="cs_b")
            exps = []
            for r in range(RT):
                if k == 4:
                    hs = 0 if r < 4 else 1
                    nsl = [hs]  # single 512-wide n-slice
                else:
                    nsl = [0, 1]
                ps = psum.tile([P, R], f32, tag="ps")
                for dc in range(DC):
                    for i, ns in enumerate(nsl):
                        nc.tensor.matmul(
                            ps[:, i * 512 : (i + 1) * 512],
                            ft[0][dc][:, r * P : (r + 1) * P],
                            ftk[dc][:, ns * 512 : (ns + 1) * 512],
                            start=(dc == 0),
                            stop=(dc == DC - 1),
                        )
                sidx = r * NK + k
                w = 512 * len(nsl)
                if k == 0:
                    anchor = nc.scalar.activation(
                        ps[:],
                        ps[:],
                        EXPF,
                        scale=1.0 / T,
                        accum_out=sums_sb[:, sidx : sidx + 1],
                    )
                else:
                    ex = exppool.tile([P, R], bf16, tag=f"exp{r}")
                    anchor = nc.scalar.activation(
                        ex[:, :w],
                        ps[:, :w],
                        EXPF,
                        scale=1.0 / T,
                        accum_out=sums_sb[:, sidx : sidx + 1],
                    )
                    exps.append((r, ex, w))
            if k >= 1:
                # Column sums via ones-matmuls accumulated across row tiles.
                if k == 4:
                    for cs, rr in ((cs_a, range(0, 4)), (cs_b, range(4, 8))):
                        for i, r in enumerate(rr):
                            _, ex, w = exps[r]
                            nc.tensor.matmul(
                                cs[:],
                                ones[:],
                                ex[:, :512],
                                start=(i == 0),
                                stop=(i == len(rr) - 1),
                            )
                    csoff = 3 * R
                    nc.vector.tensor_copy(csums_sb[:, csoff : csoff + 512], cs_a[:])
                    nc.vector.tensor_copy(
                        csums_sb[:, csoff + 512 : csoff + 1024], cs_b[:]
                    )
                else:
                    for h, cs in ((0, cs_a), (1, cs_b)):
                        for r in range(RT):
                            _, ex, _ = exps[r]
                            nc.tensor.matmul(
                                cs[:],
                                ones[:],
                                ex[:, h * 512 : (h + 1) * 512],
                                start=(r == 0),
                                stop=(r == RT - 1),
                            )
                    csoff = (k - 1) * R
                    nc.vector.tensor_copy(csums_sb[:, csoff : csoff + 512], cs_a[:])
                    nc.vector.tensor_copy(
                        csums_sb[:, csoff + 512 : csoff + 1024], cs_b[:]
                    )
            return anchor

        anchors[0] = emit_k(0, own_ft)
        for k in range(1, NK):
            _, ft[k] = build_band(k)
            anchors[k] = emit_k(k, ft[k])

        # The zpos/pos-term chain is off the matmul critical path; pin its
        # loads behind the k=1 block so they can't gap-fill into the
        # prologue and steal DMA/DVE bandwidth from the own/band-1 chain.
        from concourse.tile import add_dep_helper

        _zpos_deps = [anchors[1].ins]
        _old_gp_dma = nc.gpsimd.dma_start

        def _dep_dma(*a, **kw):
            inst = _old_gp_dma(*a, **kw)
            for dd in _zpos_deps:
                add_dep_helper(inst.ins, dd, reason="delay zpos past prologue")
            return inst

        nc.gpsimd.dma_start = _dep_dma
        pos_fns = load_norm(zpos, "pos", keepfn, "fnp")
        nc.gpsimd.dma_start = _old_gp_dma
        for s in range(NSUP):
            for a in range(ST):
                t = s * ST + a
                mulsum(
                    own_fns[s][:, a, :], pos_fns[s][:, a, :], pos_sb[:, t : t + 1]
                )

        nc.sync.dma_start(out=sums_out[:], in_=sums_sb[:])
        nc.sync.dma_start(out=csums_out[:], in_=csums_sb[:])
        nc.sync.dma_start(out=pos_out[:], in_=pos_sb[:])

    nc.compile()
    return nc


def _get_nc():
    global _NC
    if _NC is None:
        _NC = _build()
    return _NC


def run(z1, z2, trace=False):
    """Run the SPMD kernel; returns (loss, BassKernelResults)."""
    from concourse.bass_utils import run_bass_kernel_spmd

    z1 = np.ascontiguousarray(z1, dtype=np.float32)
    z2 = np.ascontiguousarray(z2, dtype=np.float32)
    zfull = np.concatenate([z1, z2], axis=0)  # [2B, D], band b = rows b*R..

    def band(b):
        return zfull[b * R : (b + 1) * R]

    in_maps = []
    for c in range(NCORES):
        sections = [band((c + k) % 8) for k in range(4)]
        b4 = band((c + 4) % 8)
        if c >= 4:
            b4 = np.concatenate([b4[512:], b4[:512]], axis=0)
        sections.append(b4)
        in_maps.append(
            {
                "zc": np.ascontiguousarray(np.concatenate(sections, axis=0)),
                "zpos": np.ascontiguousarray(band((c + 4) % 8)),
            }
        )
    res = run_bass_kernel_spmd(
        _get_nc(), in_maps, core_ids=list(range(NCORES)), trace=trace
    )

    rowtot = np.zeros(2 * B, dtype=np.float64)
    poscos = np.zeros(2 * B, dtype=np.float64)
    for c, r in enumerate(res.results):
        sums = r["sums"].astype(np.float64).reshape(P, RT, NK)
        csums = r["csums"].astype(np.float64).reshape(4 * R)
        pos = r["pos"].astype(np.float64)  # [P, RT]
        own = slice(c * R, (c + 1) * R)
        # row-sums: global row = c*R + r*128 + p
        rowtot[own] += sums.sum(axis=2).T.reshape(R)
        poscos[own] = pos.T.reshape(R)
        # col-sums k=1..3: credit rows of band (c+k)%8
        for k in (1, 2, 3):
            tgt = ((c + k) % 8) * R
            rowtot[tgt : tgt + R] += csums[(k - 1) * R : k * R]
        # k=4 quadrant col-sums: section order follows the (possibly
        # swapped) b4 input; unswap for c >= 4.
        c4 = ((c + 4) % 8) * R
        qa, qb = csums[3 * R : 3 * R + 512], csums[3 * R + 512 : 4 * R]
        if c < 4:
            rowtot[c4 : c4 + 512] += qa
            rowtot[c4 + 512 : c4 + R] += qb
        else:
            rowtot[c4 + 512 : c4 + R] += qa
            rowtot[c4 : c4 + 512] += qb
    loss = (np.log(rowtot) - poscos / T).mean()
    return np.float32(loss), res


def kernel(z1, z2, labels=None, **_ignored):
    loss, _ = run(z1, z2, trace=False)
    return np.asarray(loss, dtype=np.float32)


if __name__ == "__main__":
    rng = np.random.default_rng(0)
    a = rng.standard_normal((B, D)).astype(np.float32)
    b = rng.standard_normal((B, D)).astype(np.float32)
    print(kernel(a, b, None))
